# Initial kernel scaffold
#
"""BasesDecomposition (R-GCN style) message passing kernel for Trainium2.

Strategy (8 NeuronCores, SPMD — one program, per-core data):
  - Nodes sharded by row: core c owns targets [c*NL, (c+1)*NL).
  - Edges symmetrized on host, partitioned by target-owner core.
  - Per-relation weights W_r = sum_b rbw[r, b] * bases[b] computed on host.
  - Self-loop handled as a dense matmul with host-masked, host-transposed
    local features xm^T.
  - Phase 1 (messages): per 128-edge chunk (relation-pure, padded to a
    uniform per-relation group size G): indirect-gather x[src] rows,
    PE-transpose, matmul with W_r, write message rows sequentially to a
    DRAM buffer md. Within a relation, edges are ordered by (target
    block, rank) so md rows for one target block form one contiguous
    run per relation.
  - Phase 2 (aggregate): per 128-target block: ONE wide indirect gather
    whose 128 indices are interval starts covering the block's 32 runs
    (each index fetches SL consecutive md rows -> [128, SL*128] tile);
    for each of the SL column slices, build a one-hot*ew indicator T on
    DVE and accumulate out^T[o, t] += M_slice^T @ T on the tensor
    engine in PSUM; add the self-loop matmul W_self^T @ xm^T; store.
  - Host reassembles out from the per-core out^T blocks.
"""

import numpy as np

import concourse.bass as bass
import concourse.bacc as bacc
import concourse.tile as tile
import concourse.mybir as mybir
from concourse.bass_utils import run_bass_kernel_spmd

F32 = mybir.dt.float32
F32R = mybir.dt.float32r
I32 = mybir.dt.int32

NCORE = 8
R = 32  # num_relations (relation id R is the self-loop row of rbw)
SL_CANDIDATES = (8, 10, 12, 16)  # md rows per cover index in phase 2


def _ranks_within_group(keys, order, nbins):
    """rank of each element within its key group, following `order`."""
    counts = np.bincount(keys, minlength=nbins)
    starts = np.concatenate([[0], np.cumsum(counts)[:-1]])
    r = np.empty(len(keys), np.int64)
    r[order] = np.arange(len(keys)) - starts[keys[order]]
    return r


def host_prep(x, node_keep_mask, source, target, edge_type, edge_weights,
              bases, relation_base_weights):
    n, d = x.shape
    assert n % NCORE == 0
    nl = n // NCORE
    nblk = (nl + 127) // 128
    nlp = nblk * 128

    f32 = np.float32
    W = np.einsum("rb,bdo->rdo", relation_base_weights.astype(f32),
                  bases.astype(f32)).astype(f32)  # (R+1, 128, 128)
    wsb_h = np.ascontiguousarray(W.transpose(1, 0, 2).reshape(d, (R + 1) * d))

    src2 = np.concatenate([source, target]).astype(np.int64)
    tgt2 = np.concatenate([target, source]).astype(np.int64)
    et2 = np.concatenate([edge_type, edge_type]).astype(np.int64)
    ew2 = np.concatenate([edge_weights, edge_weights]).astype(f32)

    owner = tgt2 // nl
    tloc = tgt2 - owner * nl
    blk = tloc // 128
    tin = (tloc - blk * 128).astype(f32)

    # phase-1: uniform relation-group size G across (core, relation);
    # within a relation, order edges by target block (for phase-2 runs)
    cr = owner * R + et2
    cnt_cr = np.bincount(cr, minlength=NCORE * R)
    G = int(np.ceil(max(int(cnt_cr.max()), 1) / 128)) * 128
    ep1 = R * G
    ng1 = ep1 // 128
    order1 = np.lexsort((blk, cr))
    r1 = _ranks_within_group(cr, order1, NCORE * R)
    pos1 = et2 * G + r1  # core-local md row of each edge

    # per-(core, rel, blk) run lengths and starts (within the relation group)
    crb = cr * nblk + blk
    cnt_crb = np.bincount(crb, minlength=NCORE * R * nblk).reshape(
        NCORE, R, nblk)
    run_start = np.zeros_like(cnt_crb)
    run_start[:, :, 1:] = np.cumsum(cnt_crb, axis=2)[:, :, :-1]

    # smallest cover stride whose per-(core, block) interval count fits in
    # the 128 indices of one indirect gather
    for SL in SL_CANDIDATES:
        n_iv = np.ceil(cnt_crb / SL).sum(axis=1).max()
        if n_iv <= 128:
            break
    else:
        raise AssertionError(f"no SL fits: {n_iv} intervals")

    xf = np.ascontiguousarray(x.astype(f32))
    keep = node_keep_mask.astype(f32)

    per_core = []
    for c in range(NCORE):
        m = owner == c
        gsrc_flat = np.zeros(ep1, np.int32)
        gsrc_flat[pos1[m]] = src2[m].astype(np.int32)
        gsrc_h = np.ascontiguousarray(gsrc_flat.reshape(ng1, 128).T)

        # md row -> edge id map for this core
        edge_of_row = np.full(ep1, -1, np.int64)
        edge_ids = np.nonzero(m)[0]
        edge_of_row[pos1[edge_ids]] = edge_ids

        # phase-2 cover: per block, interval starts covering the 32 runs
        cidx_h = np.zeros((128, nblk), np.int32)
        vlen_h = np.zeros((128, nblk), np.int64)
        for b in range(nblk):
            iv = []
            for r in range(R):
                s = r * G + int(run_start[c, r, b])
                ln = int(cnt_crb[c, r, b])
                for off in range(0, ln, SL):
                    st = min(s + off, ep1 - SL)
                    iv.append((st, min(SL, s + ln - st)))
            assert len(iv) <= 128, f"cover overflow: {len(iv)} intervals"
            for p, (st, vl) in enumerate(iv):
                cidx_h[p, b] = st
                vlen_h[p, b] = vl

        # map covered rows -> (tcol, tscl) streams in cover layout
        rows = cidx_h.astype(np.int64)[:, :, None] + np.arange(SL)  # [128, nblk, SL]
        ev = edge_of_row[rows]  # [128, nblk, SL]
        in_run = np.arange(SL)[None, None, :] < vlen_h[:, :, None]
        valid = (ev >= 0) & in_run
        evc = np.where(valid, ev, 0)
        same_blk = blk[evc] == np.arange(nblk)[None, :, None]
        use = valid & same_blk
        assert int(use.sum()) == len(edge_ids), (
            f"cover mismatch: {int(use.sum())} vs {len(edge_ids)}")
        tcol_h = np.where(use, tin[evc], -1.0).astype(f32)
        tscl_h = np.where(use, ew2[evc], 0.0).astype(f32)
        tcol_h = np.ascontiguousarray(tcol_h.reshape(128, nblk * SL))
        tscl_h = np.ascontiguousarray(tscl_h.reshape(128, nblk * SL))

        xm = xf[c * nl:(c + 1) * nl] * keep[c * nl:(c + 1) * nl, None]
        xmt_h = np.zeros((128, nlp), f32)
        xmt_h[:, :nl] = xm.T

        per_core.append({
            "xg": xf,
            "wsb": wsb_h,
            "xmt": xmt_h,
            "gsrc": gsrc_h,
            "cidx": np.ascontiguousarray(cidx_h),
            "tcol": tcol_h,
            "tscl": tscl_h,
        })

    cfg = dict(n=n, nl=nl, nblk=nblk, nlp=nlp, G=G, ep1=ep1, ng1=ng1, SL=SL)
    return per_core, cfg


def build_program(cfg):
    n = cfg["n"]
    nblk = cfg["nblk"]
    nlp = cfg["nlp"]
    G = cfg["G"]
    ep1 = cfg["ep1"]
    ng1 = cfg["ng1"]
    SL = cfg["SL"]

    nc = bacc.Bacc(None, target_bir_lowering=False, debug=False)

    xg = nc.declare_dram_parameter("xg", [n, 128], F32R, isOutput=False)
    wsb = nc.declare_dram_parameter("wsb", [128, (R + 1) * 128], F32R, isOutput=False)
    xmt = nc.declare_dram_parameter("xmt", [128, nlp], F32R, isOutput=False)
    gsrc = nc.declare_dram_parameter("gsrc", [128, ng1], I32, isOutput=False)
    cidx = nc.declare_dram_parameter("cidx", [128, nblk], I32, isOutput=False)
    tcol = nc.declare_dram_parameter("tcol", [128, nblk * SL], F32, isOutput=False)
    tscl = nc.declare_dram_parameter("tscl", [128, nblk * SL], F32, isOutput=False)
    outT = nc.declare_dram_parameter("outT", [128, nlp], F32, isOutput=True)

    md = nc.dram_tensor("md", [ep1, 128], F32R)

    ident_d = nc.inline_tensor(np.eye(128, dtype=np.float32), name="ident_c")
    colidx_d = nc.inline_tensor(
        np.tile(np.arange(128, dtype=np.float32), (128, 1)), name="colidx_c")

    with tile.TileContext(nc) as tc:
        with tc.tile_pool(name="const", bufs=1) as constp:
            wsb_t = constp.tile([128, (R + 1) * 128], F32R)
            nc.sync.dma_start(out=wsb_t[:], in_=wsb[:])
            xmt_t = constp.tile([128, nlp], F32R)
            nc.sync.dma_start(out=xmt_t[:], in_=xmt[:])
            gsrc_t = constp.tile([128, ng1], I32)
            nc.sync.dma_start(out=gsrc_t[:], in_=gsrc[:])
            cidx_t = constp.tile([128, nblk], I32)
            nc.sync.dma_start(out=cidx_t[:], in_=cidx[:])
            tcol_t = constp.tile([128, nblk * SL], F32)
            nc.sync.dma_start(out=tcol_t[:], in_=tcol[:])
            tscl_t = constp.tile([128, nblk * SL], F32)
            nc.sync.dma_start(out=tscl_t[:], in_=tscl[:])
            ident_f = constp.tile([128, 128], F32)
            nc.sync.dma_start(out=ident_f[:], in_=ident_d[:])
            ident = constp.tile([128, 128], F32R)
            nc.vector.tensor_copy(out=ident[:], in_=ident_f[:])
            colidx_f = constp.tile([128, 128], F32)
            nc.sync.dma_start(out=colidx_f[:], in_=colidx_d[:])
            colidx = constp.tile([128, 128], F32R)
            nc.vector.tensor_copy(out=colidx[:], in_=colidx_f[:])

            # ---------------- Phase 1: messages ----------------
            with (
                tc.tile_pool(name="p1", bufs=20) as p1,
                tc.tile_pool(name="p1ps", bufs=3, space="PSUM") as p1ps,
            ):
                for c in range(ng1):
                    xga = p1.tile([128, 128], F32R, tag="xgather")
                    nc.gpsimd.indirect_dma_start(
                        out=xga[:], out_offset=None, in_=xg[:, :],
                        in_offset=bass.IndirectOffsetOnAxis(
                            ap=gsrc_t[:, c:c + 1], axis=0))
                    tp = p1ps.tile([128, 128], F32R, tag="tpsum")
                    nc.tensor.transpose(out=tp[:], in_=xga[:], identity=ident[:])
                    xT = p1.tile([128, 128], F32R, tag="xT")
                    nc.vector.tensor_copy(out=xT[:], in_=tp[:])
                    mp = p1ps.tile([128, 128], F32, tag="mpsum")
                    r = (c * 128) // G
                    nc.tensor.matmul(
                        out=mp[:], lhsT=xT[:],
                        rhs=wsb_t[:, 128 * r:128 * (r + 1)],
                        start=True, stop=True)
                    ms = p1.tile([128, 128], F32R, tag="mstage")
                    nc.scalar.copy(out=ms[:], in_=mp[:])
                    nc.sync.dma_start(out=md[128 * c:128 * (c + 1), :], in_=ms[:])

            # ---------------- Phase 2: aggregate ----------------
            with (
                tc.tile_pool(name="p2", bufs=10) as p2,
                tc.tile_pool(name="p2ps", bufs=4, space="PSUM") as p2ps,
            ):
                for b in range(nblk):
                    mg = p2.tile([128, SL * 128], F32R, tag="mg")
                    nc.gpsimd.indirect_dma_start(
                        out=mg[:], out_offset=None, in_=md[:, :],
                        in_offset=bass.IndirectOffsetOnAxis(
                            ap=cidx_t[:, b:b + 1], axis=0))
                    ps = p2ps.tile([128, 128], F32, tag="acc")
                    for j in range(SL):
                        tt = p2.tile([128, 128], F32R, tag="T")
                        nc.vector.tensor_scalar(
                            out=tt[:], in0=colidx[:],
                            scalar1=tcol_t[:, b * SL + j:b * SL + j + 1],
                            scalar2=tscl_t[:, b * SL + j:b * SL + j + 1],
                            op0=mybir.AluOpType.is_equal,
                            op1=mybir.AluOpType.mult)
                        nc.tensor.matmul(
                            out=ps[:],
                            lhsT=mg[:, 128 * j:128 * (j + 1)],
                            rhs=tt[:],
                            start=(j == 0), stop=False)
                    nc.tensor.matmul(
                        out=ps[:],
                        lhsT=wsb_t[:, R * 128:(R + 1) * 128],
                        rhs=xmt_t[:, 128 * b:128 * (b + 1)],
                        start=False, stop=True)
                    ob = p2.tile([128, 128], F32, tag="ob")
                    nc.vector.tensor_copy(out=ob[:], in_=ps[:])
                    nc.sync.dma_start(out=outT[:, 128 * b:128 * (b + 1)],
                                      in_=ob[:])

    nc.finalize()
    return nc


_PROGRAM_CACHE = {}


def _get_program(cfg):
    key = tuple(sorted(cfg.items()))
    if key not in _PROGRAM_CACHE:
        _PROGRAM_CACHE[key] = build_program(cfg)
    return _PROGRAM_CACHE[key]


def kernel(x, node_keep_mask, source, target, edge_type, edge_weights,
           bases, relation_base_weights):
    per_core, cfg = host_prep(x, node_keep_mask, source, target, edge_type,
                              edge_weights, bases, relation_base_weights)
    nc = _get_program(cfg)
    res = run_bass_kernel_spmd(nc, per_core, list(range(NCORE)))
    nl = cfg["nl"]
    out = np.empty((cfg["n"], 128), np.float32)
    for c in range(NCORE):
        out[c * nl:(c + 1) * nl] = res.results[c]["outT"][:, :nl].T
    return out



# revision 1
# speedup vs baseline: 1.2591x; 1.2591x over previous
"""BasesDecomposition (R-GCN style) message passing kernel for Trainium2.

Strategy (8 NeuronCores, SPMD — one program, per-core data):
  - Nodes sharded by row: core c owns targets [c*NL, (c+1)*NL).
  - Edges symmetrized on host, partitioned by target-owner core.
  - Per-relation weights W_r = sum_b rbw[r, b] * bases[b] computed on host.
  - Self-loop handled as a dense matmul with host-masked, host-transposed
    local features xm^T.
  - Phase 1 (messages): per 128-edge chunk (relation-pure, padded to a
    uniform per-relation group size G): indirect-gather x[src] rows,
    PE-transpose, matmul with W_r, write message rows sequentially to a
    DRAM buffer md. Within a relation, edges are ordered by (target
    block, rank) so md rows for one target block form one contiguous
    run per relation.
  - Phase 2 (aggregate): per 128-target block: ONE wide indirect gather
    whose 128 indices are interval starts covering the block's 32 runs
    (each index fetches SL consecutive md rows -> [128, SL*128] tile);
    for each of the SL column slices, build a one-hot*ew indicator T on
    DVE and accumulate out^T[o, t] += M_slice^T @ T on the tensor
    engine in PSUM; add the self-loop matmul W_self^T @ xm^T; store.
  - Host reassembles out from the per-core out^T blocks.
"""

import numpy as np

import concourse.bass as bass
import concourse.bacc as bacc
import concourse.tile as tile
import concourse.mybir as mybir
from concourse.bass_utils import run_bass_kernel_spmd

F32 = mybir.dt.float32
F32R = mybir.dt.float32r
I32 = mybir.dt.int32

NCORE = 8
R = 32  # num_relations (relation id R is the self-loop row of rbw)
SL_CANDIDATES = (8, 10, 12, 16)  # md rows per cover index in phase 2


def _ranks_within_group(keys, order, nbins):
    """rank of each element within its key group, following `order`."""
    counts = np.bincount(keys, minlength=nbins)
    starts = np.concatenate([[0], np.cumsum(counts)[:-1]])
    r = np.empty(len(keys), np.int64)
    r[order] = np.arange(len(keys)) - starts[keys[order]]
    return r


def host_prep(x, node_keep_mask, source, target, edge_type, edge_weights,
              bases, relation_base_weights):
    n, d = x.shape
    assert n % NCORE == 0
    nl = n // NCORE
    nblk = (nl + 127) // 128
    nlp = nblk * 128

    f32 = np.float32
    W = np.einsum("rb,bdo->rdo", relation_base_weights.astype(f32),
                  bases.astype(f32)).astype(f32)  # (R+1, 128, 128)
    wsb_h = np.ascontiguousarray(W.transpose(1, 0, 2).reshape(d, (R + 1) * d))

    src2 = np.concatenate([source, target]).astype(np.int64)
    tgt2 = np.concatenate([target, source]).astype(np.int64)
    et2 = np.concatenate([edge_type, edge_type]).astype(np.int64)
    ew2 = np.concatenate([edge_weights, edge_weights]).astype(f32)

    owner = tgt2 // nl
    tloc = tgt2 - owner * nl
    blk = tloc // 128
    tin = (tloc - blk * 128).astype(f32)

    # phase-1: uniform relation-group size G across (core, relation);
    # within a relation, order edges by target block (for phase-2 runs)
    cr = owner * R + et2
    cnt_cr = np.bincount(cr, minlength=NCORE * R)
    G = int(np.ceil(max(int(cnt_cr.max()), 1) / 128)) * 128
    ep1 = R * G
    ng1 = ep1 // 128
    order1 = np.lexsort((blk, cr))
    r1 = _ranks_within_group(cr, order1, NCORE * R)
    pos1 = et2 * G + r1  # core-local md row of each edge

    # per-(core, rel, blk) run lengths and starts (within the relation group)
    crb = cr * nblk + blk
    cnt_crb = np.bincount(crb, minlength=NCORE * R * nblk).reshape(
        NCORE, R, nblk)
    run_start = np.zeros_like(cnt_crb)
    run_start[:, :, 1:] = np.cumsum(cnt_crb, axis=2)[:, :, :-1]

    # smallest cover stride whose per-(core, block) interval count fits in
    # the 128 indices of one indirect gather
    for SL in SL_CANDIDATES:
        n_iv = np.ceil(cnt_crb / SL).sum(axis=1).max()
        if n_iv <= 128:
            break
    else:
        raise AssertionError(f"no SL fits: {n_iv} intervals")

    xf = np.ascontiguousarray(x.astype(f32))
    keep = node_keep_mask.astype(f32)

    per_core = []
    for c in range(NCORE):
        m = owner == c
        gsrc_flat = np.zeros(ep1, np.int32)
        gsrc_flat[pos1[m]] = src2[m].astype(np.int32)
        gsrc_h = np.ascontiguousarray(gsrc_flat.reshape(ng1, 128).T)

        # md row -> edge id map for this core
        edge_of_row = np.full(ep1, -1, np.int64)
        edge_ids = np.nonzero(m)[0]
        edge_of_row[pos1[edge_ids]] = edge_ids

        # phase-2 cover: per block, interval starts covering the 32 runs
        cidx_h = np.zeros((128, nblk), np.int32)
        vlen_h = np.zeros((128, nblk), np.int64)
        for b in range(nblk):
            iv = []
            for r in range(R):
                s = r * G + int(run_start[c, r, b])
                ln = int(cnt_crb[c, r, b])
                for off in range(0, ln, SL):
                    st = min(s + off, ep1 - SL)
                    iv.append((st, min(SL, s + ln - st)))
            assert len(iv) <= 128, f"cover overflow: {len(iv)} intervals"
            for p, (st, vl) in enumerate(iv):
                cidx_h[p, b] = st
                vlen_h[p, b] = vl

        # map covered rows -> (tcol, tscl) streams in cover layout
        rows = cidx_h.astype(np.int64)[:, :, None] + np.arange(SL)  # [128, nblk, SL]
        ev = edge_of_row[rows]  # [128, nblk, SL]
        in_run = np.arange(SL)[None, None, :] < vlen_h[:, :, None]
        valid = (ev >= 0) & in_run
        evc = np.where(valid, ev, 0)
        same_blk = blk[evc] == np.arange(nblk)[None, :, None]
        use = valid & same_blk
        assert int(use.sum()) == len(edge_ids), (
            f"cover mismatch: {int(use.sum())} vs {len(edge_ids)}")
        tcol_h = np.where(use, tin[evc], -1.0).astype(f32)
        tscl_h = np.where(use, ew2[evc], 0.0).astype(f32)
        tcol_h = np.ascontiguousarray(tcol_h.reshape(128, nblk * SL))
        tscl_h = np.ascontiguousarray(tscl_h.reshape(128, nblk * SL))

        xm = xf[c * nl:(c + 1) * nl] * keep[c * nl:(c + 1) * nl, None]
        xmt_h = np.zeros((128, nlp), f32)
        xmt_h[:, :nl] = xm.T

        per_core.append({
            "xg": xf,
            "wsb": wsb_h,
            "xmt": xmt_h,
            "gsrc": gsrc_h,
            "cidx": np.ascontiguousarray(cidx_h),
            "tcol": tcol_h,
            "tscl": tscl_h,
        })

    cfg = dict(n=n, nl=nl, nblk=nblk, nlp=nlp, G=G, ep1=ep1, ng1=ng1, SL=SL)
    return per_core, cfg


def build_program(cfg):
    n = cfg["n"]
    nblk = cfg["nblk"]
    nlp = cfg["nlp"]
    G = cfg["G"]
    ep1 = cfg["ep1"]
    ng1 = cfg["ng1"]
    SL = cfg["SL"]

    nc = bacc.Bacc(None, target_bir_lowering=False, debug=False)

    xg = nc.declare_dram_parameter("xg", [n, 128], F32R, isOutput=False)
    wsb = nc.declare_dram_parameter("wsb", [128, (R + 1) * 128], F32R, isOutput=False)
    xmt = nc.declare_dram_parameter("xmt", [128, nlp], F32R, isOutput=False)
    gsrc = nc.declare_dram_parameter("gsrc", [128, ng1], I32, isOutput=False)
    cidx = nc.declare_dram_parameter("cidx", [128, nblk], I32, isOutput=False)
    tcol = nc.declare_dram_parameter("tcol", [128, nblk * SL], F32, isOutput=False)
    tscl = nc.declare_dram_parameter("tscl", [128, nblk * SL], F32, isOutput=False)
    outT = nc.declare_dram_parameter("outT", [128, nlp], F32, isOutput=True)

    md = nc.dram_tensor("md", [ep1, 128], F32R)

    ident_d = nc.inline_tensor(np.eye(128, dtype=np.float32), name="ident_c")
    colidx_d = nc.inline_tensor(
        np.tile(np.arange(128, dtype=np.float32), (128, 1)), name="colidx_c")

    with tile.TileContext(nc) as tc:
        with tc.tile_pool(name="const", bufs=1) as constp:
            wsb_t = constp.tile([128, (R + 1) * 128], F32R)
            nc.sync.dma_start(out=wsb_t[:], in_=wsb[:])
            xmt_t = constp.tile([128, nlp], F32R)
            nc.sync.dma_start(out=xmt_t[:], in_=xmt[:])
            gsrc_t = constp.tile([128, ng1], I32)
            nc.sync.dma_start(out=gsrc_t[:], in_=gsrc[:])
            cidx_t = constp.tile([128, nblk], I32)
            nc.sync.dma_start(out=cidx_t[:], in_=cidx[:])
            tcol_t = constp.tile([128, nblk * SL], F32)
            nc.sync.dma_start(out=tcol_t[:], in_=tcol[:])
            tscl_t = constp.tile([128, nblk * SL], F32)
            nc.sync.dma_start(out=tscl_t[:], in_=tscl[:])
            ident_f = constp.tile([128, 128], F32)
            nc.sync.dma_start(out=ident_f[:], in_=ident_d[:])
            ident = constp.tile([128, 128], F32R)
            nc.vector.tensor_copy(out=ident[:], in_=ident_f[:])
            colidx_f = constp.tile([128, 128], F32)
            nc.sync.dma_start(out=colidx_f[:], in_=colidx_d[:])
            colidx = constp.tile([128, 128], F32R)
            nc.vector.tensor_copy(out=colidx[:], in_=colidx_f[:])

            # ---------------- Phase 1: messages ----------------
            with (
                tc.tile_pool(name="p1", bufs=20) as p1,
                tc.tile_pool(name="p1ps", bufs=3, space="PSUM") as p1ps,
            ):
                for c in range(ng1):
                    xga = p1.tile([128, 128], F32R, tag="xgather")
                    nc.gpsimd.indirect_dma_start(
                        out=xga[:], out_offset=None, in_=xg[:, :],
                        in_offset=bass.IndirectOffsetOnAxis(
                            ap=gsrc_t[:, c:c + 1], axis=0))
                    tp = p1ps.tile([128, 128], F32R, tag="tpsum")
                    nc.tensor.transpose(out=tp[:], in_=xga[:], identity=ident[:])
                    xT = p1.tile([128, 128], F32R, tag="xT")
                    nc.vector.tensor_copy(out=xT[:], in_=tp[:])
                    mp = p1ps.tile([128, 128], F32, tag="mpsum")
                    r = (c * 128) // G
                    nc.tensor.matmul(
                        out=mp[:], lhsT=xT[:],
                        rhs=wsb_t[:, 128 * r:128 * (r + 1)],
                        start=True, stop=True)
                    ms = p1.tile([128, 128], F32R, tag="mstage")
                    nc.scalar.copy(out=ms[:], in_=mp[:])
                    nc.sync.dma_start(out=md[128 * c:128 * (c + 1), :], in_=ms[:])

            # ---------------- Phase 2: aggregate ----------------
            with (
                tc.tile_pool(name="p2", bufs=10) as p2,
                tc.tile_pool(name="p2ps", bufs=4, space="PSUM") as p2ps,
            ):
                for b in range(nblk):
                    mg = p2.tile([128, SL * 128], F32R, tag="mg")
                    nc.gpsimd.indirect_dma_start(
                        out=mg[:], out_offset=None, in_=md[:, :],
                        in_offset=bass.IndirectOffsetOnAxis(
                            ap=cidx_t[:, b:b + 1], axis=0))
                    ps = p2ps.tile([128, 128], F32, tag="acc")
                    for j in range(SL):
                        tt = p2.tile([128, 128], F32R, tag="T")
                        nc.vector.tensor_scalar(
                            out=tt[:], in0=colidx[:],
                            scalar1=tcol_t[:, b * SL + j:b * SL + j + 1],
                            scalar2=tscl_t[:, b * SL + j:b * SL + j + 1],
                            op0=mybir.AluOpType.is_equal,
                            op1=mybir.AluOpType.mult)
                        nc.tensor.matmul(
                            out=ps[:],
                            lhsT=mg[:, 128 * j:128 * (j + 1)],
                            rhs=tt[:],
                            start=(j == 0), stop=False)
                    nc.tensor.matmul(
                        out=ps[:],
                        lhsT=wsb_t[:, R * 128:(R + 1) * 128],
                        rhs=xmt_t[:, 128 * b:128 * (b + 1)],
                        start=False, stop=True)
                    ob = p2.tile([128, 128], F32, tag="ob")
                    nc.vector.tensor_copy(out=ob[:], in_=ps[:])
                    nc.sync.dma_start(out=outT[:, 128 * b:128 * (b + 1)],
                                      in_=ob[:])

    nc.finalize()
    return nc


_PROGRAM_CACHE = {}


def _get_program(cfg):
    key = tuple(sorted(cfg.items()))
    if key not in _PROGRAM_CACHE:
        _PROGRAM_CACHE[key] = build_program(cfg)
    return _PROGRAM_CACHE[key]


def kernel(x, node_keep_mask, source, target, edge_type, edge_weights,
           bases, relation_base_weights):
    per_core, cfg = host_prep(x, node_keep_mask, source, target, edge_type,
                              edge_weights, bases, relation_base_weights)
    nc = _get_program(cfg)
    res = run_bass_kernel_spmd(nc, per_core, list(range(NCORE)))
    nl = cfg["nl"]
    out = np.empty((cfg["n"], 128), np.float32)
    for c in range(NCORE):
        out[c * nl:(c + 1) * nl] = res.results[c]["outT"][:, :nl].T
    return out

